# revision 1
# baseline (speedup 1.0000x reference)
"""Trainium2 Bass kernel for nn_RandProjector (histogram_binning).

Computes, for x [16384, 1024] and W [6400, 1024]:
    proj = x @ W.T                      # [S, D] -- never materialized in HBM
    per-column 20-bin histogram of proj (torch.histc semantics with
    mins/maxs as ranges), reshaped [100, 64, 20], L2-normalized over bins.

Strategy (8 NeuronCores, data-parallel over S):
  - Each core gets a 2048-row shard of x and the full W, both fp16 (host
    cast); x^T and W^T are loaded via xbar DMA-transpose (single queue --
    concurrent xbar transposes on two queues corrupt data).
  - Per 128-column tile of D: 32 fp16 matmuls accumulate proj [128, 2048]
    into PSUM (fp32); ScalarE stages it to SBUF (frees the PSUM slot in
    ~2us), then cdf_b = #(proj >= edge_b) for the 19 interior bin edges
    via fused compare+accumulate ops at 1x: 10 edges on VectorE
    (tensor_scalar is_ge), 9 on ScalarE (Sign activation).
  - Bin counts are cdf differences; bin 0 uses the constant shard total
    (mins/maxs are the true min/max of proj, so every element is in range).
  - Histogram halves AllReduce across the 8 cores as soon as their tiles
    finish (first half overlaps the second half's compute); L2-normalize
    on device; output gathered from core 0.

Edge comparisons are exact fp32 against host-precomputed edges; the only
approximation vs the fp32 reference is the fp16 rounding of the matmul
inputs (rel err ~1e-3 of a bin width; a few counts per bin).
"""

import sys

if "/opt/trn_rl_repo" not in sys.path:
    sys.path.insert(0, "/opt/trn_rl_repo")

import numpy as np

S, IN_DIM = 16384, 1024
NUM_PROJ, PROJ_DIM, BINS = 100, 64, 20
D = NUM_PROJ * PROJ_DIM          # 6400
N_CORES = 8
S_SHARD = S // N_CORES           # 2048
NE = BINS - 1                    # 19 interior edges (b = 1..19)
NV = 10                          # edges handled by VectorE (b = 1..10)
NA = NE - NV                     # edges handled by ScalarE (b = 11..19)

_CACHE = {}


def build(s_shard=S_SHARD, d=D, in_dim=IN_DIM, n_cores=N_CORES, debug=False):
    import concourse.bacc as bacc
    import concourse.bass as bass
    from concourse import mybir
    from concourse.tile import TileContext

    f32 = mybir.dt.float32
    f16 = mybir.dt.float16
    nt = d // 128
    kc_n = in_dim // 128
    chw = min(512, s_shard)      # matmul moving-operand width
    nch_n = s_shard // chw

    nc = bacc.Bacc("TRN2", target_bir_lowering=False, debug=False,
                   num_devices=n_cores)

    xs_d = nc.dram_tensor("xs16", [s_shard, in_dim], f16, kind="ExternalInput")
    w_d = nc.dram_tensor("w16", [d, in_dim], f16, kind="ExternalInput")
    edges_d = nc.dram_tensor("edges", [128, nt, NV], f32, kind="ExternalInput")
    nedges_d = nc.dram_tensor("nedges", [128, nt, NA], f32, kind="ExternalInput")
    out_d = nc.dram_tensor("out", [d, BINS], f32, kind="ExternalOutput")
    # pieces of the histogram all-reduce independently, overlapping compute
    if nt >= 8:
        pieces = [(0, 2 * nt // 5), (2 * nt // 5, 4 * nt // 5), (4 * nt // 5, nt)]
    else:
        pieces = [(0, nt)]
    cc_ins, cc_outs = [], []
    for i, (t0, t1) in enumerate(pieces):
        cc_ins.append(nc.dram_tensor(f"cc_in{i}", [128, (t1 - t0) * BINS], f32))
        cc_outs.append(nc.dram_tensor(f"cc_out{i}", [128, (t1 - t0) * BINS],
                                      f32, addr_space="Shared"))
    if debug:
        dbg_hist = nc.dram_tensor("dbg_hist", [128, nt, BINS], f32,
                                  kind="ExternalOutput")
        dbg_cdf = nc.dram_tensor("dbg_cdf", [128, nt, BINS + 1], f32,
                                 kind="ExternalOutput")

    with TileContext(nc) as tc:
        with (
            tc.tile_pool(name="singles", bufs=1) as singles,
            tc.tile_pool(name="sp_pool", bufs=3) as sp_pool,
            tc.tile_pool(name="ps_p", bufs=2, space="PSUM") as ps_p,
        ):
            edges = singles.tile([128, nt, NV], f32)
            nedges = singles.tile([128, nt, NA], f32)
            nc.sync.dma_start(out=edges, in_=edges_d[:, :, :])
            nc.sync.dma_start(out=nedges, in_=nedges_d[:, :, :])

            trash_v = singles.tile([128, s_shard], f16)
            trash_a = singles.tile([128, s_shard], f16)

            # per-engine cdf accumulators (separate tiles so Tile never
            # serializes VectorE against ScalarE on writes)
            acc_v = singles.tile([128, nt, NV], f32)
            acc_a = singles.tile([128, nt, NA], f32)

            # ---- Phase 0: DMA-transpose x shard and W into SBUF ----
            # One DMA queue only: concurrent xbar transposes on two queues
            # corrupt data. x first, then W in d-chunks (first chunk small
            # so tile 0 isn't gated on the whole load).
            xT = singles.tile([128, kc_n, s_shard], f16)
            wT = singles.tile([128, kc_n, d], f16)
            for kc in range(kc_n):
                nc.sync.dma_start_transpose(
                    out=xT[:, kc, :], in_=xs_d[:, kc * 128:(kc + 1) * 128])
            d_bounds = [0]
            while d_bounds[-1] < d:
                nxt = 256 if d_bounds[-1] == 0 else 800
                d_bounds.append(min(d_bounds[-1] + nxt, d))
            for d0, d1 in zip(d_bounds[:-1], d_bounds[1:]):
                for kc in range(kc_n):
                    nc.sync.dma_start_transpose(
                        out=wT[:, kc, d0:d1],
                        in_=w_d[d0:d1, kc * 128:(kc + 1) * 128])

            # normalization scratch (allocated up front, used per half)
            cdfx = singles.tile([128, nt, BINS + 1], f32)
            nc.vector.memset(cdfx[:, :, 0:1], float(s_shard))
            nc.vector.memset(cdfx[:, :, BINS:BINS + 1], 0.0)
            hist = singles.tile([128, nt, BINS], f32)
            hsum = singles.tile([128, nt, BINS], f32)
            sq = singles.tile([128, nt, BINS], f32)
            n2 = singles.tile([128, nt], f32)
            y_t = singles.tile([128, nt], f32)
            iy = singles.tile([128, nt], f32)
            a_t = singles.tile([128, nt], f32)
            b_t = singles.tile([128, nt], f32)
            r_t = singles.tile([128, nt], f32)
            outn = singles.tile([128, nt, BINS], f32)
            out_v = out_d[:, :].rearrange("(t p) b -> p t b", p=128)

            def emit_cc(hi):
                """Combine cdf partials for tau in [t0, t1) and kick off the
                cross-core all-reduce (runs on DMA/CC queues in background)."""
                t0, t1 = pieces[hi]
                sl = slice(t0, t1)
                nc.vector.tensor_copy(cdfx[:, sl, 1:1 + NV], acc_v[:, sl])
                # ScalarE counts are sums of sign in {-1,0,1}:
                # cdf = 0.5*sgn + N/2
                nc.vector.tensor_scalar(
                    cdfx[:, sl, 1 + NV:BINS], acc_a[:, sl],
                    0.5, float(s_shard) / 2,
                    op0=mybir.AluOpType.mult, op1=mybir.AluOpType.add)
                nc.vector.tensor_tensor(
                    out=hist[:, sl], in0=cdfx[:, sl, 0:BINS],
                    in1=cdfx[:, sl, 1:BINS + 1],
                    op=mybir.AluOpType.subtract)
                if debug and hi == len(pieces) - 1:
                    nc.sync.dma_start(out=dbg_hist[:, :, :], in_=hist)
                    nc.sync.dma_start(out=dbg_cdf[:, :, :], in_=cdfx)
                nc.sync.dma_start(
                    out=cc_ins[hi][:, :],
                    in_=hist[:, sl].rearrange("p a b -> p (a b)"))
                nc.gpsimd.collective_compute(
                    "AllReduce",
                    mybir.AluOpType.add,
                    replica_groups=[list(range(n_cores))],
                    ins=[cc_ins[hi][:, :]],
                    outs=[cc_outs[hi][:, :]],
                )
                nc.sync.dma_start(
                    out=hsum[:, sl].rearrange("p a b -> p (a b)"),
                    in_=cc_outs[hi][:, :])

            hsum_g = singles.tile([128, nt, BINS], f32)

            def emit_norm(t0, t1, guard):
                """L2-normalize the summed histogram for tau in [t0, t1) and
                write the output slice. When `guard`, route hsum through a
                no-op add of last-tile accumulator data so the scheduler's
                cost model places the chain after the final tile -- the
                collective is then long finished and no engine FIFO stalls
                on it."""
                sl = slice(t0, t1)
                w = t1 - t0
                if guard:
                    g_ap = acc_a[:, nt - 1, NA - 1:NA]
                    g_b = bass.AP(tensor=g_ap.tensor, offset=g_ap.offset,
                                  ap=[g_ap.ap[0], [0, w], [0, BINS]])
                    nc.vector.scalar_tensor_tensor(
                        out=hsum_g[:, sl], in0=g_b, scalar=0.0,
                        in1=hsum[:, sl],
                        op0=mybir.AluOpType.mult, op1=mybir.AluOpType.add)
                    h_in = hsum_g
                else:
                    h_in = hsum
                nc.vector.tensor_tensor(out=sq[:, sl], in0=h_in[:, sl],
                                        in1=h_in[:, sl],
                                        op=mybir.AluOpType.mult)
                nc.vector.tensor_reduce(out=n2[:, sl], in_=sq[:, sl],
                                        axis=mybir.AxisListType.X,
                                        op=mybir.AluOpType.add)
                nc.scalar.sqrt(y_t[:, sl], n2[:, sl])
                nc.vector.reciprocal(iy[:, sl], y_t[:, sl])
                # one Newton step for rsqrt: r = iy * (1.5 - 0.5*n2*iy^2)
                nc.vector.tensor_tensor(out=a_t[:, sl], in0=iy[:, sl],
                                        in1=iy[:, sl],
                                        op=mybir.AluOpType.mult)
                nc.vector.tensor_tensor(out=b_t[:, sl], in0=a_t[:, sl],
                                        in1=n2[:, sl],
                                        op=mybir.AluOpType.mult)
                nc.vector.tensor_scalar(b_t[:, sl], b_t[:, sl], -0.5, 1.5,
                                        op0=mybir.AluOpType.mult,
                                        op1=mybir.AluOpType.add)
                nc.vector.tensor_tensor(out=r_t[:, sl], in0=iy[:, sl],
                                        in1=b_t[:, sl],
                                        op=mybir.AluOpType.mult)
                r_ap = r_t[:, sl]
                r_b = bass.AP(tensor=r_ap.tensor, offset=r_ap.offset,
                              ap=[r_ap.ap[0], r_ap.ap[1], [0, BINS]])
                nc.vector.tensor_tensor(out=outn[:, sl], in0=h_in[:, sl],
                                        in1=r_b, op=mybir.AluOpType.mult)
                nc.sync.dma_start(out=out_v[:, sl], in_=outn[:, sl])

            # ---- Phase 1: d-tiles ----
            for tau in range(nt):
                pp = ps_p.tile([128, s_shard], f32)
                for nch in range(nch_n):
                    for kc in range(kc_n):
                        nc.tensor.matmul(
                            pp[:, nch * chw:(nch + 1) * chw],
                            lhsT=wT[:, kc, tau * 128:(tau + 1) * 128],
                            rhs=xT[:, kc, nch * chw:(nch + 1) * chw],
                            start=(kc == 0),
                            stop=(kc == kc_n - 1),
                        )
                # Stage PSUM -> SBUF once (frees the PSUM slot for the next
                # tile's matmuls after ~2us instead of ~21us)
                sp = sp_pool.tile([128, s_shard], f32)
                nc.scalar.copy(out=sp, in_=pp)
                # ScalarE: edges b = NV+1 .. 19 via Sign(p - edge)
                for j in range(NA):
                    nc.scalar.activation(
                        out=trash_a,
                        in_=sp,
                        func=mybir.ActivationFunctionType.Sign,
                        bias=nedges[:, tau, j:j + 1],
                        scale=1.0,
                        accum_out=acc_a[:, tau, j:j + 1],
                    )
                # VectorE: edges b = 1 .. NV via is_ge
                for j in range(NV):
                    nc.vector.tensor_scalar(
                        trash_v,
                        sp,
                        edges[:, tau, j:j + 1],
                        None,
                        op0=mybir.AluOpType.is_ge,
                        op1=mybir.AluOpType.add,
                        accum_out=acc_v[:, tau, j:j + 1],
                    )
                for hi, (t0, t1) in enumerate(pieces):
                    if tau == t1 - 1:
                        emit_cc(hi)
            # all normalization at the end: earlier pieces' collectives have
            # long completed (guarded so the scheduler knows); only the last
            # piece's collective is actually waited on
            if len(pieces) > 1:
                emit_norm(0, pieces[-2][1], guard=True)
            emit_norm(pieces[-1][0], pieces[-1][1], guard=False)

    nc.compile()
    return nc


def host_prep(x, W, mins, maxs, s_shard=S_SHARD, n_cores=N_CORES):
    d = W.shape[0]
    nt = d // 128
    x16 = np.asarray(x, dtype=np.float16)
    w16 = np.ascontiguousarray(np.asarray(W, dtype=np.float16))
    mins64 = np.asarray(mins, dtype=np.float64)
    maxs64 = np.asarray(maxs, dtype=np.float64)
    b = np.arange(1, BINS, dtype=np.float64)          # 19 interior edges
    edges = (mins64[:, None] + (maxs64 - mins64)[:, None] * (b[None, :] / BINS))
    edges = edges.astype(np.float32)                   # [d, 19]
    edges_l = edges.reshape(nt, 128, NE).transpose(1, 0, 2)  # [128, nt, 19]
    edges_dev = np.ascontiguousarray(edges_l[:, :, :NV])
    nedges_dev = np.ascontiguousarray(-edges_l[:, :, NV:])
    in_maps = []
    for i in range(n_cores):
        in_maps.append({
            "xs16": np.ascontiguousarray(x16[i * s_shard:(i + 1) * s_shard]),
            "w16": w16,
            "edges": edges_dev,
            "nedges": nedges_dev,
        })
    return in_maps


def run(x, W, mins, maxs, trace=False, **trace_kw):
    """Returns (output [100, 64, 20] f32, BassKernelResults)."""
    from concourse.bass_utils import run_bass_kernel_spmd

    if "nc" not in _CACHE:
        _CACHE["nc"] = build()
    nc = _CACHE["nc"]
    in_maps = host_prep(x, W, mins, maxs)
    res = run_bass_kernel_spmd(nc, in_maps, core_ids=list(range(N_CORES)),
                               trace=trace, **trace_kw)
    out = res.results[0]["out"].reshape(NUM_PROJ, PROJ_DIM, BINS)
    return np.asarray(out, dtype=np.float32), res


def kernel(x, W, mins, maxs, num_of_projection=NUM_PROJ, bins=BINS):
    assert int(num_of_projection) == NUM_PROJ and int(bins) == BINS
    out, _ = run(x, W, mins, maxs, trace=False)
    return out

